# revision 33
# baseline (speedup 1.0000x reference)
"""Trainium2 Bass kernel for nn_CustomLstm (D=2048, H=1024), 8-core tensor-parallel.

Sharding: all five weights/biases and outputs are sharded along the units (row)
dimension of W across 8 NeuronCores (256 rows each).  The (D,D) concat
activation is replicated; gate elementwise ops are local; ht is all-gathered
(in 4 column chunks, fp8) so the final w5 @ ht matmul + row softmax is local.

Precision plan (validated against the fp32 reference in numpy simulation,
worst rel err ~1.1e-2 vs the 2e-2 gate):
 - Gates 1/2/4/5 matmuls in fp8 e4m3 with DoubleRow perf mode (2x bf16 PE
   throughput, half the weight/activation DMA bytes).
 - Gate 3 (tanh candidate gate, error slope 1.0 -> dominates ct error) stays
   bf16: w3 bf16, concat streamed in bf16 for it.
 - Biases in fp8 e3m4 scaled x16 host-side; the x(1/16) unscale is fused into
   the DVE bias-add via scalar_tensor_tensor.
 - ct/ht outputs in bf16 (upcast to f32 on host), yt output f32.
 - ht all-gathered in e4m3 for the DoubleRow w5 matmul.

DMA plan: batch everything into >=0.5 MB transfers on the two HWDGE rings
(sync/scalar); gpsimd SWDGE (~1-2 us fixed cost per op) only carries small
latency-tolerant output writes.  Biases/cprev load once per rep as whole
[128, 2, D] tiles; the fp8 concat copy for chunks 1-3 is converted on the DVE
from the bf16 stream instead of being DMA'd.
"""

import numpy as np
import ml_dtypes

import concourse.bass as bass
import concourse.bacc as bacc
import concourse.mybir as mybir
import concourse.tile as tile
import concourse.bass_utils as bass_utils

BF16 = ml_dtypes.bfloat16
E4 = ml_dtypes.float8_e4m3
E3 = ml_dtypes.float8_e3m4

D = 2048          # units == input dim of each weight matrix
N_CORES = 8
R = D // N_CORES  # 256 rows per core
PK = D // 128     # 16 contraction chunks of 128
PJ = PK // 2      # 8 DoubleRow pair chunks of 256
NN = 4            # 4 column chunks of 512
NCOL = D // NN    # 512
NM = R // 128     # 2 row chunks of 128
BSCALE = 16.0     # host-side bias scale (for e3m4 range); unscaled on DVE

_CACHE = None


def _build(reps=1, single=False, fake_ag=False):
    nc = bacc.Bacc("TRN2", target_bir_lowering=False, debug=False,
                   num_devices=1 if single else N_CORES)
    f32 = mybir.dt.float32
    bf16 = mybir.dt.bfloat16
    fp8 = mybir.dt.float8e4
    fp8b = mybir.dt.float8e3
    AF = mybir.ActivationFunctionType
    ALU = mybir.AluOpType
    DR = mybir.MatmulPerfMode.DoubleRow

    concat = nc.dram_tensor("concat", [D, D], bf16, kind="ExternalInput").ap()
    # fp8 weights (1/2/4/5), host-packed in DoubleRow pair layout
    # [128, (j, m, two, 128)]
    w8 = {g: nc.dram_tensor(f"w{g}p", [128, PJ * NM * 2 * 128], fp8,
                            kind="ExternalInput").ap()
          for g in (1, 2, 4, 5)}
    w3t = nc.dram_tensor("w3t", [D, R], bf16, kind="ExternalInput").ap()
    b = [nc.dram_tensor(f"b{g}", [R, D], fp8b, kind="ExternalInput").ap()
         for g in range(1, 6)]
    cprev = nc.dram_tensor("cprev", [R, D], bf16, kind="ExternalInput").ap()

    ct_o = nc.dram_tensor("ct_o", [R, D], bf16, kind="ExternalOutput").ap()
    ht_o = nc.dram_tensor("ht_o", [R, D], bf16, kind="ExternalOutput").ap()
    yt_o = nc.dram_tensor("yt_o", [R, D], f32, kind="ExternalOutput").ap()

    rg = [list(range(N_CORES))]

    with tile.TileContext(nc) as tc:
        with (
            tc.tile_pool(name="wpool", bufs=1) as wpool,
            tc.tile_pool(name="xpool", bufs=2) as xpool,
            tc.tile_pool(name="hpool", bufs=2) as hpool,
            tc.tile_pool(name="gpool", bufs=1) as gpool,
            tc.tile_pool(name="zpool", bufs=1) as zpool,
            tc.tile_pool(name="spool", bufs=4) as spool,
            tc.tile_pool(name="psum", bufs=1, space="PSUM") as pp,
            tc.tile_pool(name="dram", bufs=1, space="DRAM") as dram,
        ):
            for rep in range(reps):
                # fp8 DoubleRow weights: [p, j, m, two, c]; w1 double-buffered
                # so the next rep's load overlaps this rep's phase A
                wp_sb = {g: wpool.tile([128, PJ, NM, 2, 128], fp8,
                                       name=f"w{g}p", tag=f"w{g}p",
                                       bufs=2 if g == 1 else 1)
                         for g in (1, 2, 4, 5)}
                w3_sb = wpool.tile([128, PK * R], bf16, name="w3sb", tag="w3sb")
                # biases / cprev as whole per-rep tiles: [p, m, D]
                bt_sb = [wpool.tile([128, NM, D], fp8b, name=f"bt{g}",
                                    tag=f"bt{g}", bufs=2 if g == 1 else 1)
                         for g in range(1, 6)]
                cp_sb = wpool.tile([128, NM, D], bf16, name="cp", tag="cp")
                w_loaded = {g: False for g in (1, 2, 3, 4, 5)}

                def load_w(g):
                    if w_loaded[g]:
                        return
                    w_loaded[g] = True
                    if g == 3:
                        # 1 MB bf16 load on the SP ring
                        nc.sync.dma_start(
                            w3_sb[:].rearrange("p (k m) -> p k m", m=R),
                            w3t.rearrange("(k p) m -> p k m", p=128))
                    else:
                        nc.scalar.dma_start(
                            wp_sb[g][:].rearrange("p j m t c -> p (j m t c)"),
                            w8[g])

                def load_b(g):
                    nc.scalar.dma_start(
                        bt_sb[g - 1][:],
                        b[g - 1].rearrange("(m p) d -> p m d", p=128))

                # w1 + first concat chunk first so the PE starts immediately;
                # then interleave bias/weight loads on the ACT ring in the
                # order phase A consumes them (gate order on chunk 0 is
                # 1, 2, 4, 3; w2 rides the gpsimd ring)
                load_w(1)
                load_w(2)
                load_b(1)

                x_tiles = [None] * NN

                def issue_x(n):
                    csl = slice(n * NCOL, (n + 1) * NCOL)
                    x_sb = xpool.tile([128, PK, NCOL], bf16, name=f"x{n}",
                                      tag="xsb")
                    x8_sb = xpool.tile([128, PK, NCOL], fp8, name=f"x8{n}",
                                       tag="x8sb")
                    x_tiles[n] = (x_sb, x8_sb)
                    if n == 0:
                        # feed the first chunk from the SP ring in halves;
                        # the fp8 copy is converted on the DVE per half so
                        # the first DoubleRow matmuls start after half the
                        # bytes (saves the 1 MB concat8 DMA entirely)
                        hk = PK // 2
                        for h in range(2):
                            ksl = slice(h * hk, (h + 1) * hk)
                            nc.sync.dma_start(
                                x_sb[:, ksl, :],
                                concat[h * D // 2:(h + 1) * D // 2, csl]
                                .rearrange("(k p) c -> p k c", p=128))
                            nc.vector.tensor_copy(x8_sb[:, ksl, :],
                                                  x_sb[:, ksl, :])
                    else:
                        # later chunks: bf16 via DMA; the fp8 conversion is
                        # issued separately (conv_x) so its DVE slot can be
                        # placed where it doesn't stall the elementwise chain
                        # (saves 1 MB of HBM traffic per chunk)
                        eng = nc.scalar if n == 2 else nc.sync
                        eng.dma_start(
                            x_sb[:],
                            concat[:, csl].rearrange("(k p) c -> p k c", p=128))

                def conv_x(n):
                    x_sb, x8_sb = x_tiles[n]
                    nc.vector.tensor_copy(x8_sb[:], x_sb[:])

                # --- phase C per-chunk body (interleaved into phase A) ---
                exs = [zpool.tile([128, D], f32, name=f"ex{m}", tag=f"ex{m}")
                       for m in range(NM)]
                sms = [[spool.tile([128, 1], f32, name=f"sm{m}_{n}",
                                   tag=f"sm{m}_{n}") for n in range(NN)]
                       for m in range(NM)]
                ag_outs = []

                h_tiles = [None] * NN

                def phase_c_read(q):
                    h_sb = hpool.tile([128, PK, NCOL], fp8, name=f"h{q}",
                                      tag=f"hsb{q}", bufs=1)
                    h_tiles[q] = h_sb
                    # one batched readback DMA per all-gather output piece;
                    # alternate rings per chunk, and the split last chunk
                    # rides both HWDGE rings in parallel
                    for i, (ago, coff, cw) in enumerate(ag_outs[q]):
                        eng = nc.scalar if (q + i) % 2 else nc.sync
                        eng.dma_start(
                            h_sb[:, :, coff:coff + cw],
                            ago.rearrange("(k p) c -> p k c", p=128))

                def phase_c_chunk(q):
                    csl = slice(q * NCOL, (q + 1) * NCOL)
                    h_sb = h_tiles[q]
                    for m in range(NM):
                        p5 = pp.tile([128, NCOL], f32, name="ps5", tag="ps5",
                                     bufs=2)
                        for j in range(PJ):
                            nc.tensor.matmul(
                                p5[:],
                                wp_sb[5][:, j, m],
                                h_sb[:, 2 * j:2 * j + 2, :],
                                start=(j == 0), stop=(j == PJ - 1),
                                perf_mode=DR)
                        z5c = gpool.tile([128, NCOL], f32, name="z5c",
                                         tag="z5c", bufs=2)
                        nc.vector.scalar_tensor_tensor(
                            z5c[:], bt_sb[4][:, m, csl], 1.0 / BSCALE,
                            p5[:], ALU.mult, ALU.add)
                        # chunked exp with per-chunk row-sum so only the last
                        # 512-col exp sits in the kernel tail; logits are
                        # bounded (|z| < ~1: 0.05-scale inputs), so exp
                        # without max subtraction is safe
                        nc.scalar.activation(exs[m][:, csl], z5c[:],
                                             AF.Exp, accum_out=sms[m][q][:])

                issue_x(0)
                # --- phase A: gates, ct, ht; all-gather ht by column chunk ---
                for n in range(NN):
                    csl = slice(n * NCOL, (n + 1) * NCOL)
                    x_sb, x8_sb = x_tiles[n]
                    if n == 0:
                        load_w(4)
                        load_b(2)
                        load_b(4)
                        load_w(5)
                        nc.scalar.dma_start(
                            cp_sb[:], cprev.rearrange("(m p) d -> p m d",
                                                      p=128))
                        load_b(3)
                        load_b(5)
                    if n + 1 < NN:
                        issue_x(n + 1)

                    asp = "Local" if (single or fake_ag) else "Shared"
                    if n == NN - 1 and not (single or fake_ag):
                        h2 = NCOL // 2
                        ag_in = [dram.tile([R, h2], fp8, name=f"agin{n}{s}",
                                           tag=f"agin{n}{s}")
                                 for s in ("a", "b")]
                        ag_out = [dram.tile([D, h2], fp8, name=f"agout{n}{s}",
                                            tag=f"agout{n}{s}", addr_space=asp)
                                  for s in ("a", "b")]
                        ag_outs.append([(ag_out[0], 0, h2),
                                        (ag_out[1], h2, h2)])
                    else:
                        ag_in = dram.tile([R, NCOL], fp8, name=f"agin{n}",
                                          tag=f"agin{n}")
                        ag_out = dram.tile([D, NCOL], fp8, name=f"agout{n}",
                                           tag=f"agout{n}", addr_space=asp)
                        ag_outs.append([(ag_out, 0, NCOL)])

                    # order gates by weight-arrival; gate 3 (bf16, big w3
                    # load) goes last on the first chunk
                    gate_order = [1, 2, 4, 3] if n == 0 else [1, 2, 3, 4]
                    for m in range(NM):
                        rsl = slice(m * 128, (m + 1) * 128)
                        ps = {}
                        for g in gate_order:
                            load_w(g)
                            p = pp.tile([128, NCOL], f32, name=f"ps{g}",
                                        tag=f"ps{g}",
                                        bufs=2 if g == 1 else 1)
                            if g == 3:
                                for k in range(PK):
                                    nc.tensor.matmul(
                                        p[:],
                                        w3_sb[:, k * R + m * 128:
                                              k * R + (m + 1) * 128],
                                        x_sb[:, k, :],
                                        start=(k == 0), stop=(k == PK - 1))
                            else:
                                for j in range(PJ):
                                    nc.tensor.matmul(
                                        p[:],
                                        wp_sb[g][:, j, m],
                                        x8_sb[:, 2 * j:2 * j + 2, :],
                                        start=(j == 0), stop=(j == PJ - 1),
                                        perf_mode=DR)
                            ps[g] = p
                        load_w(5)  # w5 queued early; needed only in phase C

                        acts = {}
                        for g, fn in [(1, AF.Sigmoid), (2, AF.Sigmoid),
                                      (3, AF.Tanh), (4, AF.Sigmoid)]:
                            pre = gpool.tile([128, NCOL], f32, name=f"pre{g}",
                                             tag=f"pre{g}")
                            nc.vector.scalar_tensor_tensor(
                                pre[:], bt_sb[g - 1][:, m, csl], 1.0 / BSCALE,
                                ps[g][:], ALU.mult, ALU.add)
                            act = gpool.tile([128, NCOL], f32, name=f"act{g}",
                                             tag=f"act{g}")
                            nc.scalar.activation(act[:], pre[:], fn)
                            acts[g] = act

                        t1 = gpool.tile([128, NCOL], f32, name="t1", tag="t1")
                        nc.vector.tensor_mul(t1[:], acts[1][:],
                                             cp_sb[:, m, csl])
                        t2 = gpool.tile([128, NCOL], f32, name="t2", tag="t2")
                        nc.vector.tensor_mul(t2[:], acts[2][:], acts[3][:])
                        ctt = gpool.tile([128, NCOL], bf16, name="ctt",
                                         tag="ctt", bufs=2)
                        nc.vector.tensor_add(ctt[:], t1[:], t2[:])
                        nc.gpsimd.dma_start(ct_o[rsl, csl], ctt[:])

                        th = gpool.tile([128, NCOL], f32, name="th", tag="th")
                        nc.scalar.activation(th[:], ctt[:], AF.Tanh)
                        htt = gpool.tile([128, NCOL], bf16, name="htt",
                                         tag="htt", bufs=2)
                        nc.vector.tensor_mul(htt[:], acts[4][:], th[:])
                        nc.gpsimd.dma_start(ht_o[rsl, csl], htt[:])
                        # fp8 copy for the all-gather, produced directly on
                        # the DVE so the scalar engine isn't a serial hop
                        htb = gpool.tile([128, NCOL], fp8, name="htb",
                                         tag="htb", bufs=2)
                        nc.vector.tensor_mul(htb[:], acts[4][:], th[:])
                        if isinstance(ag_in, list):
                            h2 = NCOL // 2
                            nc.gpsimd.dma_start(ag_in[0][rsl, :], htb[:, :h2])
                            nc.gpsimd.dma_start(ag_in[1][rsl, :], htb[:, h2:])
                        else:
                            nc.gpsimd.dma_start(ag_in[rsl, :], htb[:])
                        if m == 0 and n + 1 < NN:
                            # chunk n+1's fp8 conversion sits between the two
                            # m-blocks' DVE work: its input DMA has landed by
                            # now, so it fills DVE slack without delaying the
                            # m0 chain that feeds ct/ht/all-gather
                            conv_x(n + 1)

                    if single or fake_ag:
                        # stand-in for the AllGather: model only the
                        # dependency chain (the real collective's data moves
                        # on dedicated SDMA rings, not the HWDGE rings)
                        agos = ag_out if isinstance(ag_out, list) else [ag_out]
                        agis = ag_in if isinstance(ag_in, list) else [ag_in]
                        for agi, ago in zip(agis, agos):
                            nc.sync.dma_start(ago[:R, :], agi[:])
                    elif isinstance(ag_in, list):
                        for agi, ago in zip(ag_in, ag_out):
                            nc.gpsimd.collective_compute(
                                "AllGather", mybir.AluOpType.bypass,
                                replica_groups=rg,
                                ins=[agi.opt()], outs=[ago.opt()])
                    else:
                        nc.gpsimd.collective_compute(
                            "AllGather", mybir.AluOpType.bypass,
                            replica_groups=rg,
                            ins=[ag_in.opt()], outs=[ag_out.opt()])

                    # issue the readback DMA for this chunk right behind its
                    # all-gather: the data lands during later phase A chunks,
                    # so the tail matmuls never wait on DMA (the matmuls
                    # themselves stay in the tail -- putting them here would
                    # head-of-line-block the PE queue on collective latency)
                    phase_c_read(n)

                # --- phase C tail: w5 matmuls + exp, then row softmax ---
                for q in range(NN):
                    phase_c_chunk(q)

                for m in range(NM):
                    s01 = spool.tile([128, 1], f32, name="s01", tag="s01")
                    nc.vector.tensor_add(s01[:], sms[m][0][:], sms[m][1][:])
                    s23 = spool.tile([128, 1], f32, name="s23", tag="s23")
                    nc.vector.tensor_add(s23[:], sms[m][2][:], sms[m][3][:])
                    sm_t = spool.tile([128, 1], f32, name="sm_t", tag="sm_t")
                    nc.vector.tensor_add(sm_t[:], s01[:], s23[:])
                    rs = spool.tile([128, 1], f32, name="rs", tag="rs")
                    nc.vector.reciprocal(rs[:], sm_t[:])
                    ex = exs[m]
                    for j in range(NN):
                        jsl = slice(j * NCOL, (j + 1) * NCOL)
                        nc.vector.tensor_scalar_mul(ex[:, jsl], ex[:, jsl],
                                                    rs[:])
                        nc.gpsimd.dma_start(yt_o[m * 128:(m + 1) * 128,
                                                 jsl], ex[:, jsl])

    nc.compile()
    return nc


_RUNNER = None


def _build_runner(nc):
    """Cached jit-compiled SPMD executor mirroring run_bass_kernel_spmd's
    axon/PJRT path, so repeat kernel() calls skip retracing."""
    import jax
    from jax.sharding import Mesh, PartitionSpec, NamedSharding
    from jax.experimental.shard_map import shard_map
    from concourse.bass2jax import (_bass_exec_p, install_neuronx_cc_hook,
                                    partition_id_tensor)

    install_neuronx_cc_hook()
    partition_name = (nc.partition_id_tensor.name
                      if nc.partition_id_tensor else None)
    in_names, out_names, out_avals = [], [], []
    for alloc in nc.m.functions[0].allocations:
        if not isinstance(alloc, mybir.MemoryLocationSet):
            continue
        name = alloc.memorylocations[0].name
        if alloc.kind == "ExternalInput":
            if name != partition_name:
                in_names.append(name)
        elif alloc.kind == "ExternalOutput":
            out_names.append(name)
            out_avals.append(jax.core.ShapedArray(
                tuple(alloc.tensor_shape), mybir.dt.np(alloc.dtype)))
    n_params, n_outs = len(in_names), len(out_names)
    all_in = tuple(in_names + out_names
                   + ([partition_name] if partition_name else []))

    def _body(*args):
        operands = list(args)
        if partition_name is not None:
            operands.append(partition_id_tensor())
        return tuple(_bass_exec_p.bind(
            *operands, out_avals=tuple(out_avals), in_names=all_in,
            out_names=tuple(out_names), lowering_input_output_aliases=(),
            sim_require_finite=True, sim_require_nnan=True, nc=nc))

    devices = jax.devices()[:N_CORES]
    mesh = Mesh(np.asarray(devices), ("core",))
    specs = (PartitionSpec("core"),) * (n_params + n_outs)
    fn = jax.jit(
        shard_map(_body, mesh=mesh, in_specs=specs,
                  out_specs=(PartitionSpec("core"),) * n_outs,
                  check_rep=False),
        donate_argnums=tuple(range(n_params, n_params + n_outs)),
        keep_unused=True)
    sh = NamedSharding(mesh, PartitionSpec("core"))
    zeros = [np.zeros((N_CORES * av.shape[0], *av.shape[1:]), av.dtype)
             for av in out_avals]

    def run(in_maps):
        gin = [jax.device_put(
            np.concatenate([in_maps[c][nm] for c in range(N_CORES)], 0), sh)
            for nm in in_names]
        gz = [jax.device_put(z, sh) for z in zeros]
        out = fn(*gin, *gz)
        got = {nm: np.asarray(o) for nm, o in zip(out_names, out)}
        return [{nm: got[nm].reshape(N_CORES, -1, got[nm].shape[-1])[c]
                 for nm in out_names} for c in range(N_CORES)]

    return run


def _pack_dr(wg):
    """Pack a (R, D) weight slice into the DoubleRow SBUF layout
    [128, (j, m, two, 128)] where k-chunk pairs (2j, 2j+1) sit adjacent."""
    wT = np.ascontiguousarray(np.asarray(wg, np.float32).T)  # [D, R] = [k, m]
    a = wT.reshape(PJ, 2, 128, NM, 128)                  # [j, t, p, m, c]
    return np.ascontiguousarray(
        a.transpose(2, 0, 3, 1, 4).reshape(128, PJ * NM * 2 * 128)).astype(E4)


def _make_in_maps(inputs):
    inp = {k: np.asarray(v) for k, v in inputs.items()}
    concat = np.concatenate([inp["hPrev"], inp["xt"]], axis=0)
    concat_bf = concat.astype(BF16)
    in_maps = []
    for i in range(N_CORES):
        r = slice(i * R, (i + 1) * R)
        m = {"concat": concat_bf,
             "cprev": np.ascontiguousarray(inp["cPrev"][r]).astype(BF16),
             "w3t": np.ascontiguousarray(inp["w3"][r].T).astype(BF16)}
        for g in (1, 2, 4, 5):
            m[f"w{g}p"] = _pack_dr(inp[f"w{g}"][r])
        for g in range(1, 6):
            m[f"b{g}"] = np.ascontiguousarray(
                inp[f"b{g}"][r] * BSCALE).astype(E3)
        in_maps.append(m)
    return in_maps


def kernel(**inputs):
    global _CACHE, _RUNNER
    if _CACHE is None:
        _CACHE = _build()
    nc = _CACHE
    in_maps = _make_in_maps(inputs)

    results = None
    if _RUNNER is not False:
        try:
            if _RUNNER is None:
                _RUNNER = _build_runner(nc)
            results = _RUNNER(in_maps)
        except Exception:
            _RUNNER = False  # fall back permanently for this process
    if results is None:
        res = bass_utils.run_bass_kernel_spmd(nc, in_maps,
                                              core_ids=list(range(N_CORES)))
        results = res.results

    ct = np.concatenate([results[i]["ct_o"] for i in range(N_CORES)], 0)
    ht = np.concatenate([results[i]["ht_o"] for i in range(N_CORES)], 0)
    yt = np.concatenate([results[i]["yt_o"] for i in range(N_CORES)], 0)
    return (ct.astype(np.float32), ht.astype(np.float32),
            yt.astype(np.float32))
